# revision 24
# baseline (speedup 1.0000x reference)
"""Trainium2 Bass kernel for SimCLR NT-Xent contrastive loss (N=4096, D=512, T=0.5).

Math: with z = rownorm(concat(emb_i, emb_j)) (8192x512) and S = z @ z.T:
  denom_r = rowsum_r(exp(S/T)) - e^2;  loss = mean_r[log denom_r - 2*S[r, r+N mod 2N]]

Distribution (symmetric, data-parallel): 8 blocks of 1024 rows. Core c gets the
5120-row window starting at row 1024c (host np.roll slice) and computes
exp-blocks against its own block L0 = rows [0:1024):
  - (L0, L1..L3): full 1024x1024 blocks -> row-sums (denom partials of L0 rows,
    via ACT exp accum; blocks 1+2 share one [128,2048] PSUM tile and one EXP)
    and column-sums (denom partials of L1..L3 rows, via fp8 DoubleRow
    ones-matmul over stacked m-tile pairs of exp values).
  - (L0, L0) and (L0, L4): "banded" blocks over 128-col tiles at tile distance
    dt = (jt - it) mod 8 in {0..4}: rows summed at full weight for dt 0..4;
    col-sums only for dt 1..3.  Summed over all 8 cores this counts every
    unordered pair exactly once (the self-paired classes dt=0 and dt=4 are
    computed by both involved cores, each contributing its own rows).
Host merges per-row partial denominators, subtracts e^2, takes log, adds the
positive-pair terms (per-row dots computed on-device).

Numerics: z stored as fp8e4 scaled by 64 (dots = 4096*sim; exp scale 2^-11).
Matmuls run fp8 DoubleRow (2 contraction planes of 128 -> 2x PE throughput);
zT built by a u16 DMA-transpose of byte-paired fp8 + a deinterleave copy pass
into plane-major layout (DoubleRow requires >=16B plane stride).
"""

import numpy as np

for _p in ("/opt/trn_rl_repo", "/root/.axon_site/_ro/trn_rl_repo"):
    try:
        import concourse  # noqa: F401
        break
    except ImportError:
        import sys
        if _p not in sys.path:
            sys.path.insert(0, _p)

import concourse.bass as bass
import concourse.bacc as bacc
import concourse.tile as tile
from concourse import mybir
from concourse.bass_utils import run_bass_kernel_spmd

F32 = mybir.dt.float32
I32 = mybir.dt.int32
U16 = mybir.dt.uint16
BF16 = mybir.dt.bfloat16
FP8 = mybir.dt.float8e4
ALU = mybir.AluOpType
AF = mybir.ActivationFunctionType
DR = mybir.MatmulPerfMode.DoubleRow

N_CORES = 8
BATCH = 4096
DIM = 512
ROWS = 2 * BATCH
BLOCK = 1024
NBLK = 5
WROWS = NBLK * BLOCK
P = 128
MT = BLOCK // P             # 8 m-tiles per block
BAND = 5 * P                # banded width per m-tile (dt 0..4)
MAGIC = 0x5F3759DF
EXPSC = 2.0 / 4096.0
N_DUMMY = 40

# how many of the 40 square-accum slices run on ACT instead of DVE (balance)
SQ_ON_ACT = 0


def _band_pieces(m):
    """Banded m-tile: band-relative psum cols [0:640) -> zT cols
    [(128m + q) mod 1024], split at the mod-1024 wrap and at psum col 512."""
    pieces = []
    dst = 0
    while dst < BAND:
        src = (m * P + dst) % BLOCK
        lim = min(BAND - dst, BLOCK - src)
        if dst < 512:
            lim = min(lim, 512 - dst)
        pieces.append((dst, src, lim))
        dst += lim
    return pieces


def _build_program():
    nc = bacc.Bacc(trn_type="TRN2")
    x_in = nc.declare_dram_parameter("x", [WROWS, DIM], F32, isOutput=False)
    drow_out = nc.declare_dram_parameter("drow", [P, MT], F32, isOutput=True)
    dcol_out = nc.declare_dram_parameter("dcol", [1, NBLK * BLOCK], F32,
                                         isOutput=True)
    pos_out = nc.declare_dram_parameter("pos", [P, MT], F32, isOutput=True)

    with tile.TileContext(nc) as tc:
        with tc.tile_pool(name="xg", bufs=10) as xg_pool, \
             tc.tile_pool(name="small", bufs=2) as small_pool, \
             tc.tile_pool(name="sq", bufs=2) as sq_pool, \
             tc.tile_pool(name="zt", bufs=6) as zt_pool, \
             tc.tile_pool(name="eb", bufs=16) as eb_pool, \
             tc.tile_pool(name="e12", bufs=4) as e12_pool, \
             tc.tile_pool(name="e3", bufs=4) as e3_pool, \
             tc.tile_pool(name="single", bufs=1) as singles, \
             tc.tile_pool(name="zdram", bufs=1, space="DRAM") as dram_pool, \
             tc.tile_pool(name="psum", bufs=2, space="PSUM") as psum_pool:

            def W(us):
                tc.tile_set_cur_wait(us / 1000.0)

            n2 = singles.tile([P, NBLK * MT], F32, tag="n2")
            inv = singles.tile([P, NBLK * MT], F32, tag="inv")
            magic8 = singles.tile([P, MT], I32, tag="magic8")
            nc.vector.memset(magic8, MAGIC)
            accm = singles.tile([P, MT, 4], F32, tag="accm")
            nc.vector.memset(accm.rearrange("p a b -> p (a b)"), 0.0)
            pos_acc = singles.tile([P, MT], F32, tag="pos_acc")
            onesf = singles.tile([P, 16], FP8, tag="onesf")
            nc.vector.memset(onesf, 1.0)
            onesf2 = singles.tile([P, 2, 16], FP8, tag="onesf2")
            nc.vector.memset(onesf2.rearrange("p a b -> p (a b)"), 1.0)
            onesb = singles.tile([P, 16], BF16, tag="onesb")
            nc.vector.memset(onesb, 1.0)
            drhs = singles.tile([P, 256], BF16, tag="drhs")
            nc.vector.memset(drhs, 0.0)
            colout = singles.tile([1, NBLK * BLOCK], F32, tag="colout")

            zbh = [[singles.tile([P, 4, DIM], FP8, tag=f"zb{b}_{h}",
                                 name=f"zb{b}_{h}") for h in range(2)]
                   for b in range(NBLK)]
            zTs = [[singles.tile([P, 2, BLOCK], FP8, tag=f"zs{b}_{k}",
                                 name=f"zs{b}_{k}") for k in range(2)]
                   for b in range(NBLK)]
            # block 0 split per half (head latency); blocks 1-4 whole
            zd0h = [dram_pool.tile([512, DIM], FP8, tag=f"zd0_{h}",
                                   name=f"zd0_{h}") for h in range(2)]
            zdw = [None] + [dram_pool.tile([BLOCK, DIM], FP8, tag=f"zd{b}",
                                           name=f"zdw{b}")
                            for b in range(1, NBLK)]

            # ---- input loads: just-in-time on SP so the DMA queues never
            # hold more than ~3MB ahead of the consumer. partition-major:
            # partition p holds 4 consecutive rows (8KB per descriptor) ----
            xgs = [xg_pool.tile([P, 4, DIM], F32, tag="xg", name=f"xg{i//2}_{i%2}")
                   for i in range(2 * NBLK)]
            load_cursor = [0]

            def emit_load():
                i = load_cursor[0]
                if i >= 2 * NBLK:
                    return
                load_cursor[0] += 1
                b, h = i // 2, i % 2
                W(1.0 + 2.2 * i)
                nc.sync.dma_start(
                    out=xgs[i],
                    in_=x_in[b * BLOCK + h * 512:
                             b * BLOCK + (h + 1) * 512, :].rearrange(
                        "(p a) d -> p a d", p=P))

            for _ in range(3):
                emit_load()
            W(0.2)

            # ---- PE warmup dummies ----
            dps = psum_pool.tile([P, 2048], F32, tag="ps", name="dps")
            for i in range(N_DUMMY):
                nc.tensor.matmul(dps[0:16, 0:256], lhsT=onesb[:, 0:16],
                                 rhs=drhs, start=True, stop=True,
                                 skip_group_check=True)

            e_tiles = {}
            pending_deint = []
            sq_count = [0]

            def flush_deint(n=None):
                todo = pending_deint[:n] if n else list(pending_deint)
                del pending_deint[:len(todo)]
                for (dst, src, eng) in todo:
                    if eng == 0:
                        nc.vector.tensor_scalar(
                            out=dst, in0=src, scalar1=1.0, scalar2=None,
                            op0=ALU.mult)
                    else:
                        nc.scalar.copy(out=dst, in_=src)

            def norm_block(b):
                """rownorm 1024 rows, cast fp8*64, write zd, transpose; queue
                deinterleave copies (flushed one block later)."""
                # squares (f32) -> n2, per half
                for h in range(2):
                    W(4.4 + 4.4 * b + 2.2 * h)
                    xg = xgs[b * 2 + h]
                    for a in range(4):
                        g = b * MT + h * 4 + a
                        sqt = sq_pool.tile([P, DIM], F32, tag="sq")
                        sq_count[0] += 1
                        eng = nc.scalar if sq_count[0] <= SQ_ON_ACT else None
                        if eng is None:
                            nc.vector.scalar_tensor_tensor(
                                out=sqt, in0=xg[:, a, :], scalar=1.0 / 4096.0,
                                in1=xg[:, a, :], op0=ALU.mult, op1=ALU.mult,
                                accum_out=n2[:, g:g + 1])
                        else:
                            nc.scalar.activation(
                                out=sqt, in_=xg[:, a, :], func=AF.Square,
                                scale=1.0 / 64.0,
                                accum_out=n2[:, g:g + 1])
                # rsqrt via Quake seed + 2 Newton steps -> 64/||x||
                W(9.2 + 4.4 * b)
                sl = n2[:, b * MT:(b + 1) * MT]
                isl = inv[:, b * MT:(b + 1) * MT]
                sh = small_pool.tile([P, MT], I32, tag="sh")
                nc.vector.tensor_scalar(
                    out=sh, in0=sl.bitcast(I32), scalar1=1, scalar2=None,
                    op0=ALU.logical_shift_right)
                seed = small_pool.tile([P, MT], I32, tag="seed")
                nc.vector.scalar_tensor_tensor(
                    out=seed, in0=magic8, scalar=0.0, in1=sh,
                    op0=ALU.bypass, op1=ALU.subtract)
                y = seed.bitcast(F32)
                for it in range(2):
                    ta = small_pool.tile([P, MT], F32, tag="ta")
                    tb = small_pool.tile([P, MT], F32, tag="tb")
                    nc.vector.tensor_mul(out=ta, in0=y, in1=y)
                    nc.vector.scalar_tensor_tensor(
                        out=tb, in0=ta, scalar=-0.5, in1=sl,
                        op0=ALU.mult, op1=ALU.mult)
                    nc.vector.tensor_scalar(
                        out=tb, in0=tb, scalar1=1.5, scalar2=None, op0=ALU.add)
                    dst = isl if it == 1 else y
                    nc.vector.tensor_mul(out=dst, in0=y, in1=tb)
                for h in range(2):
                    W(10.4 + 4.4 * b + 1.6 * h)
                    xg = xgs[b * 2 + h]
                    for a in range(4):
                        g = b * MT + h * 4 + a
                        nc.vector.tensor_scalar_mul(
                            out=zbh[b][h][:, a, :], in0=xg[:, a, :],
                            scalar1=inv[:, g:g + 1])
                    zd_dst = (zd0h[h][:, :] if b == 0 else
                              zdw[b][h * 512:(h + 1) * 512, :])
                    W(12.0 + 4.4 * b + 1.8 * h)
                    nc.sync.dma_start(
                        out=zd_dst.rearrange("(p s) d -> p s d", p=P),
                        in_=zbh[b][h])
                    if b == 0:
                        zdu = zd0h[h].bitcast(U16)
                        W(13.2 + 1.8 * h)
                        for kp in range(2):
                            zt = zt_pool.tile([P, 512], U16, tag="zt",
                                              name=f"zt0_{h}_{kp}")
                            nc.sync.dma_start_transpose(
                                out=zt, in_=zdu[:, kp * P:(kp + 1) * P])
                            ztf = zt.bitcast(FP8).rearrange(
                                "p (r two) -> p two r", two=2)
                            for j in range(2):
                                dst = zTs[0][kp][:, j,
                                                 h * 512:(h + 1) * 512]
                                pending_deint.append(
                                    (dst, ztf[:, j, :], (kp + j) % 2))
                    emit_load()
                if b > 0:
                    zdu = zdw[b].bitcast(U16)
                    W(14.8 + 4.4 * b)
                    for kp in range(2):
                        ztb = zt_pool.tile([P, BLOCK], U16, tag="ztb",
                                           name=f"zt{b}_{kp}")
                        nc.sync.dma_start_transpose(
                            out=ztb, in_=zdu[:, kp * P:(kp + 1) * P])
                        ztf = ztb.bitcast(FP8).rearrange(
                            "p (r two) -> p two r", two=2)
                        for j in range(2):
                            pending_deint.append(
                                (zTs[b][kp][:, j, :], ztf[:, j, :],
                                 (kp + j) % 2))

            def mains_banded(b):
                slot = 0 if b == 0 else 1
                for m in range(MT):
                    ps = psum_pool.tile([P, 2048], F32, tag="ps")
                    for (dst, src, ln) in _band_pieces(m):
                        for kp in range(2):
                            nc.tensor.matmul(
                                ps[:, dst:dst + ln],
                                lhsT=zTs[0][kp][:, :, m * P:(m + 1) * P],
                                rhs=zTs[b][kp][:, :, src:src + ln],
                                start=(kp == 0), stop=(kp == 1), perf_mode=DR)
                    et = eb_pool.tile([P, BAND], FP8, tag="eb")
                    e_tiles[(b, m)] = et
                    nc.scalar.activation(
                        out=et, in_=ps[:, 0:BAND], func=AF.Exp, scale=EXPSC,
                        accum_out=accm[:, m, slot:slot + 1])

            def mains_pair12():
                """blocks 1 and 2 share a [128, 2048] psum tile + one EXP."""
                for m in range(MT):
                    ps = psum_pool.tile([P, 2048], F32, tag="ps")
                    for blk in (1, 2):
                        for half in range(2):
                            c0 = (blk - 1) * 1024 + half * 512
                            for kp in range(2):
                                nc.tensor.matmul(
                                    ps[:, c0:c0 + 512],
                                    lhsT=zTs[0][kp][:, :, m * P:(m + 1) * P],
                                    rhs=zTs[blk][kp][:, :, half * 512:
                                                     (half + 1) * 512],
                                    start=(kp == 0), stop=(kp == 1),
                                    perf_mode=DR)
                    if m % 2 == 0:
                        et = e12_pool.tile([P, 2, 2048], FP8, tag="ep")
                        e_tiles[(12, m // 2)] = et
                    else:
                        et = e_tiles[(12, m // 2)]
                    nc.scalar.activation(
                        out=et[:, m % 2, :], in_=ps, func=AF.Exp, scale=EXPSC,
                        accum_out=accm[:, m, 2:3])

            def mains_b3():
                for m in range(MT):
                    ps = psum_pool.tile([P, 2048], F32, tag="ps")
                    for half in range(2):
                        c0 = half * 512
                        for kp in range(2):
                            nc.tensor.matmul(
                                ps[:, c0:c0 + 512],
                                lhsT=zTs[0][kp][:, :, m * P:(m + 1) * P],
                                rhs=zTs[3][kp][:, :, half * 512:
                                               (half + 1) * 512],
                                start=(kp == 0), stop=(kp == 1), perf_mode=DR)
                    if m % 2 == 0:
                        et = e3_pool.tile([P, 2, 1024], FP8, tag="e3")
                        e_tiles[(3, m // 2)] = et
                    else:
                        et = e_tiles[(3, m // 2)]
                    nc.scalar.activation(
                        out=et[:, m % 2, :], in_=ps[:, 0:1024], func=AF.Exp,
                        scale=EXPSC, accum_out=accm[:, m, 3:4])

            def cols_banded(b):
                """burst: banded col-sums dt 1..3 into cp[0:1, 0:1024);
                one PSUM start/stop per 2KB bank."""
                cp = psum_pool.tile([P, 2048], F32, tag="ps",
                                    name=f"cpb{b}")
                for m in range(MT):
                    et = e_tiles[(b, m)]
                    for dt in (1, 2, 3):
                        jc = (m + dt) % MT
                        bank = jc // 4
                        st = (m, dt) == ((0, 1) if bank == 0 else (1, 3))
                        sp = (m, dt) == ((7, 3) if bank == 0 else (6, 1))
                        nc.tensor.matmul(
                            cp[0:1, jc * P:(jc + 1) * P],
                            lhsT=onesf[:, 0:1],
                            rhs=et[:, dt * P:(dt + 1) * P],
                            start=st, stop=sp, skip_group_check=True)
                nc.vector.tensor_scalar(
                    out=colout[0:1, b * BLOCK:(b + 1) * BLOCK],
                    in0=cp[0:1, 0:BLOCK], scalar1=1.0, scalar2=None,
                    op0=ALU.mult)
                for m in range(MT):
                    e_tiles.pop((b, m), None)

            def cols_pair12():
                cp = psum_pool.tile([P, 2048], F32, tag="ps", name="cp12")
                for mp in range(4):
                    et = e_tiles[(12, mp)]
                    for q in range(4):   # 4 x 512-col chunks over 2 blocks
                        nc.tensor.matmul(
                            cp[0:1, q * 512:(q + 1) * 512],
                            lhsT=onesf2[:, :, 0:1],
                            rhs=et[:, :, q * 512:(q + 1) * 512],
                            start=(mp == 0), stop=(mp == 3),
                            perf_mode=DR, skip_group_check=True)
                nc.vector.tensor_scalar(
                    out=colout[0:1, BLOCK:3 * BLOCK],
                    in0=cp[0:1, 0:2048], scalar1=1.0, scalar2=None,
                    op0=ALU.mult)
                for mp in range(4):
                    e_tiles.pop((12, mp), None)

            def cols_b3():
                cp = psum_pool.tile([P, 2048], F32, tag="ps", name="cp3")
                for mp in range(4):
                    et = e_tiles[(3, mp)]
                    for q in range(2):
                        nc.tensor.matmul(
                            cp[0:1, q * 512:(q + 1) * 512],
                            lhsT=onesf2[:, :, 0:1],
                            rhs=et[:, :, q * 512:(q + 1) * 512],
                            start=(mp == 0), stop=(mp == 3),
                            perf_mode=DR, skip_group_check=True)
                nc.vector.tensor_scalar(
                    out=colout[0:1, 3 * BLOCK:4 * BLOCK],
                    in0=cp[0:1, 0:BLOCK], scalar1=1.0, scalar2=None,
                    op0=ALU.mult)
                for mp in range(4):
                    e_tiles.pop((3, mp), None)

            # ---- emission (manual stage times via W) ----
            norm_block(0)
            norm_block(1)
            W(15.5)
            flush_deint(8)         # block 0
            W(17.0)
            mains_banded(0)
            W(20.5)
            flush_deint(4)         # block 1
            norm_block(2)
            W(25.0)
            flush_deint(4)         # block 2
            W(26.0)
            mains_pair12()
            W(28.0)
            cols_banded(0)
            norm_block(3)
            W(30.0)
            flush_deint(4)         # block 3
            W(31.0)
            mains_b3()
            W(33.0)
            cols_pair12()
            norm_block(4)
            W(34.5)
            flush_deint(4)         # block 4
            # pos dots: z_L0[i] . z_L4[i] (raw, x4096 scale)
            W(35.5)
            for s in range(MT):
                psc = sq_pool.tile([P, DIM], BF16, tag="psc")
                nc.vector.scalar_tensor_tensor(
                    out=psc, in0=zbh[0][s // 4][:, s % 4, :], scalar=0.0,
                    in1=zbh[4][s // 4][:, s % 4, :], op0=ALU.bypass,
                    op1=ALU.mult, accum_out=pos_acc[:, s:s + 1])
            W(34.8)
            mains_banded(4)
            W(37.0)
            cols_b3()
            W(40.0)
            cols_banded(4)

            # ---- outputs ----
            W(44.0)
            drow = singles.tile([P, MT], F32, tag="drow")
            for m in range(MT):
                nc.vector.reduce_sum(
                    out=drow[:, m:m + 1], in_=accm[:, m, :],
                    axis=mybir.AxisListType.X)
            nc.sync.dma_start(out=drow_out[:, :], in_=drow)
            nc.sync.dma_start(out=dcol_out[:, :], in_=colout)
            nc.sync.dma_start(out=pos_out[:, :], in_=pos_acc)

    nc.finalize()
    return nc


_CACHE = {}


def _run(full: np.ndarray, trace: bool = False, **kwargs):
    if "nc" not in _CACHE:
        _CACHE["nc"] = _build_program()
    nc = _CACHE["nc"]
    in_maps = []
    for c in range(N_CORES):
        idx0 = (c * BLOCK) % ROWS
        win = np.concatenate([full[idx0:], full[:idx0]], axis=0)[:WROWS]
        in_maps.append({"x": np.ascontiguousarray(win)})
    return run_bass_kernel_spmd(
        nc, in_maps, core_ids=list(range(N_CORES)), trace=trace, **kwargs)


def _merge(results) -> np.ndarray:
    den = np.zeros(ROWS, dtype=np.float64)
    pos = np.zeros(ROWS, dtype=np.float64)
    # pos_acc[p, s] = row (s//4)*512 + 4p + (s%4)  (partition-major loads)
    s_idx = np.repeat(np.arange(MT), P)
    p_idx = np.tile(np.arange(P), MT)
    pos_rows = (s_idx // 4) * 512 + 4 * p_idx + (s_idx % 4)
    for c, r in enumerate(results):
        rows0 = np.arange(BLOCK) + BLOCK * c
        den[rows0] += r["drow"].astype(np.float64).T.reshape(-1)
        pos[BLOCK * c + pos_rows] = r["pos"].astype(np.float64).T.reshape(-1) / 4096.0
        dcol = r["dcol"].astype(np.float64).reshape(NBLK, BLOCK)
        for j in range(NBLK):
            rows_j = (np.arange(BLOCK) + BLOCK * ((c + j) % N_CORES)) % ROWS
            den[rows_j] += dcol[j]
    denom = den - np.exp(2.0)
    loss = np.mean(np.log(denom) - 2.0 * pos)
    return np.array(loss, dtype=np.float32)


def kernel(emb_i: np.ndarray, emb_j: np.ndarray) -> np.ndarray:
    full = np.concatenate(
        [np.asarray(emb_i, np.float32), np.asarray(emb_j, np.float32)], axis=0)
    return _merge(_run(full).results)
